# revision 13
# baseline (speedup 1.0000x reference)
"""Box-SDF (CAPUDF box boundary distance) Trainium2 Bass kernel, v5.

For each 3-D point x (S = 0.4), with a = |x| (host-computed; the SDF is
sign-symmetric):
    q  = a - S
    d  = sqrt(sum_i relu(q_i)^2)    if any q_i >= 0   (outside)
    d  = -max_i q_i                 otherwise         (inside)

Select-free identity used on chip: the relu(q_0) plane and the inside
term (min(max_i a_i, S) - S) are never simultaneously nonzero, so they
merge exactly into ONE signed plane (squaring kills the sign):
    e0 = max(a_0 - S, min(max(a_1, a_2), S) - S)
         (= relu(q_0) outside, = max_i a_i - S < 0 inside)
    d  = sqrt( e0^2 + relu(q_1)^2 + relu(q_2)^2 )

On-chip dataflow per tile (planar f16 input [P, 3K], all contiguous;
pre = [e0 | b1 | b2], sq = pre^2 elementwise, split ACT/DVE at column U):
    DVE: q0  = TS(a0, add -S)                    (4x-mode tensor_scalar)
         b12 = TS([a1|a2], max S, add -S)
         m12 = TT(a1, a2, max)                   (2x-mode tensor_tensor)
         mc  = TS(m12, min S, add -S)
         e0  = TT(q0, mc, max)
         sq[U:3K]  = TT(pre * pre) -> bf16
    ACT: sq[0:U]   = Square(pre[0:U]) -> bf16    (one pass, no bias)
         d    = Sqrt(s_psum) -> f16              (same activation table set)
    PE : s = sq_e0 + sq1 + sq2 via identity-matmul PSUM accumulation
         (3 planes x K/512 chunks; eye stationary in bf16)
Tile sizes [512, 2048, ..., 1024, 512] shorten the pipeline head
(first compute starts after a 0.5 MB DMA) and tail (small last B-stage).
f16/bf16 end-to-end halves HBM traffic vs f32; data-parallel on 8 cores.
"""

import sys

import numpy as np

sys.path.insert(0, "/opt/trn_rl_repo")

import concourse.bacc as bacc  # noqa: E402
import concourse.mybir as mybir  # noqa: E402
from concourse import bass_utils  # noqa: E402
from concourse.tile import TileContext  # noqa: E402

N = 8388608
NCORES = 8
NPC = N // NCORES  # 1,048,576 points per core
P = 128
KS = [512, 2048, 2048, 2048, 1024, 512]  # points per partition row, per tile
NT = len(KS)
assert P * sum(KS) == NPC

SIZE = 0.4
F16 = mybir.dt.float16
BF16 = mybir.dt.bfloat16
F32 = mybir.dt.float32
AF = mybir.ActivationFunctionType
OP = mybir.AluOpType


def build_kernel():
    nc = bacc.Bacc(
        "TRN2",
        target_bir_lowering=False,
        debug=False,
        num_devices=NCORES,
    )
    xs = [
        nc.dram_tensor(f"x{t}", [P, 3 * k], F16, kind="ExternalInput").ap()
        for t, k in enumerate(KS)
    ]
    eye = nc.dram_tensor("eye", [P, P], BF16, kind="ExternalInput").ap()
    ds = [
        nc.dram_tensor(f"d{t}", [P, k], F16, kind="ExternalOutput").ap()
        for t, k in enumerate(KS)
    ]

    with TileContext(nc) as tc:
        with (
            tc.tile_pool(name="const", bufs=1) as cpool,
            tc.tile_pool(name="xtp", bufs=4) as xtp,
            tc.tile_pool(name="pre", bufs=3) as prep,
            tc.tile_pool(name="sq", bufs=3) as sqp,
            tc.tile_pool(name="small", bufs=3) as small,
            tc.tile_pool(name="out", bufs=3) as outp,
            tc.tile_pool(name="psum", bufs=2, space="PSUM") as pspool,
        ):
            eye_t = cpool.tile([P, P], BF16)
            # Warm the Square/Sqrt activation table set while DMA ramps up.
            warm = cpool.tile([P, 8], F16)
            nc.vector.memset(warm[:], 0.0)
            nc.scalar.activation(out=warm[:], in_=warm[:], func=AF.Square)
            nc.scalar.activation(out=warm[:], in_=warm[:], func=AF.Sqrt)
            state = {}

            def stage_a(t):
                K = KS[t]
                xt = xtp.tile([P, 3 * K], F16, tag="xt")
                if t == 0:
                    # Chunk tile 0's DMA per plane so DVE starts sooner.
                    for c in range(3):
                        cs = slice(c * K, (c + 1) * K)
                        nc.sync.dma_start(out=xt[:, cs], in_=xs[t][:, cs])
                else:
                    nc.sync.dma_start(out=xt[:], in_=xs[t])

                a0, a1, a2 = (xt[:, c * K : (c + 1) * K] for c in range(3))
                U = 2 * K  # ACT squares pre[0:U]; DVE squares pre[U:3K]
                # pre = [e0 | b1 | b2]
                pre = prep.tile([P, 3 * K], F16, tag="pre")
                # q0 = a0 - S (signed)
                q0 = small.tile([P, K], F16, tag="q0")
                nc.vector.tensor_scalar(
                    out=q0[:], in0=a0, scalar1=-SIZE, scalar2=None, op0=OP.add
                )
                # b12 = relu(a12 - S) = max(a12, S) - S
                nc.vector.tensor_scalar(
                    out=pre[:, K : 3 * K],
                    in0=xt[:, K : 3 * K],
                    scalar1=SIZE,
                    scalar2=-SIZE,
                    op0=OP.max,
                    op1=OP.add,
                )
                # m12 = max(a1, a2); mc = min(m12, S) - S
                m12 = small.tile([P, K], F16, tag="m12")
                nc.vector.tensor_tensor(out=m12[:], in0=a1, in1=a2, op=OP.max)
                mc = small.tile([P, K], F16, tag="mc")
                nc.vector.tensor_scalar(
                    out=mc[:],
                    in0=m12[:],
                    scalar1=SIZE,
                    scalar2=-SIZE,
                    op0=OP.min,
                    op1=OP.add,
                )
                # e0 = max(q0, mc): relu(q0) outside, max_i a_i - S inside
                nc.vector.tensor_tensor(
                    out=pre[:, 0:K], in0=q0[:], in1=mc[:], op=OP.max
                )

                # sq = pre^2 in bf16 (full-rate PE moving data):
                # ACT squares [0:U], DVE squares [U:3K]
                sq = sqp.tile([P, 3 * K], BF16, tag="sq")
                nc.vector.tensor_tensor(
                    out=sq[:, U : 3 * K],
                    in0=pre[:, U : 3 * K],
                    in1=pre[:, U : 3 * K],
                    op=OP.mult,
                )
                nc.scalar.activation(
                    out=sq[:, 0:U],
                    in_=pre[:, 0:U],
                    func=AF.Square,
                )
                state[t] = sq

            def stage_b(t):
                K = KS[t]
                sq = state.pop(t)
                # s = sq_e0 + sq1 + sq2 via identity matmuls accumulating in
                # PSUM (TensorE is otherwise idle; accumulate = free add)
                s_ps = pspool.tile([P, K], F32, tag="s_ps")
                dt = outp.tile([P, K], F16, tag="dt")
                last = t >= NT - 2
                for j in range(0, K, 512):
                    for c in range(3):
                        nc.tensor.matmul(
                            s_ps[:, j : j + 512],
                            eye_t[:],
                            sq[:, c * K + j : c * K + j + 512],
                            start=(c == 0),
                            stop=(c == 2),
                        )
                    if last:
                        # Tail tile: sqrt + store per 512-chunk so the
                        # final DMA overlaps the remaining matmul groups.
                        js = slice(j, j + 512)
                        nc.scalar.activation(
                            out=dt[:, js], in_=s_ps[:, js], func=AF.Sqrt
                        )
                        nc.gpsimd.dma_start(out=ds[t][:, js], in_=dt[:, js])
                if not last:
                    # d = sqrt(s)  (ScalarE reads PSUM directly, writes f16)
                    nc.scalar.activation(out=dt[:], in_=s_ps[:], func=AF.Sqrt)
                    nc.gpsimd.dma_start(out=ds[t], in_=dt[:])

            # 2-stage software pipeline emission: A(t+1) before B(t) so each
            # engine's in-order stream never stalls tile t+1's front work
            # behind tile t's tail work.
            stage_a(0)
            nc.sync.dma_start(out=eye_t[:], in_=eye[:])
            for t in range(1, NT):
                stage_b(t - 1)
                stage_a(t)
            stage_b(NT - 1)

    nc.compile()
    return nc


_cached_nc = None


def _get_nc():
    global _cached_nc
    if _cached_nc is None:
        _cached_nc = build_kernel()
    return _cached_nc


_AXON_SO = "/opt/axon/libaxon_pjrt.so"


def _ensure_ntff_hook():
    """Install an antenv.axon_hooks shim backed by libaxon_pjrt's NRT
    profiling C ABI, so run_bass_kernel_spmd(trace=True) works under axon."""
    try:
        from antenv.axon_hooks import get_axon_ntff_profile_hook  # noqa: F401

        return
    except ImportError:
        pass
    import contextlib
    import ctypes
    import types

    import antenv

    holder = {}
    mod = types.ModuleType("antenv.axon_hooks")
    mod.set_axon_ntff_profile_hook = lambda h: holder.__setitem__("h", h)
    mod.get_axon_ntff_profile_hook = lambda: holder.get("h")
    sys.modules["antenv.axon_hooks"] = mod
    antenv.axon_hooks = mod

    try:
        lib = ctypes.CDLL(_AXON_SO)
    except OSError:
        return
    if not hasattr(lib, "axon_start_nrt_profile"):
        return
    lib.axon_start_nrt_profile.argtypes = [
        ctypes.POINTER(ctypes.c_int64),
        ctypes.c_size_t,
    ]
    lib.axon_start_nrt_profile.restype = ctypes.c_int64
    lib.axon_stop_nrt_profile.argtypes = [ctypes.c_char_p]
    lib.axon_stop_nrt_profile.restype = ctypes.c_int64

    @contextlib.contextmanager
    def _hook(output_dir, device_ids):
        import jax

        jax.devices()
        if device_ids:
            ids = (ctypes.c_int64 * len(device_ids))(*device_ids)
            rc = lib.axon_start_nrt_profile(ids, len(device_ids))
        else:
            rc = lib.axon_start_nrt_profile(None, 0)
        if rc != 0:
            raise RuntimeError(f"axon_start_nrt_profile rc={rc}")
        try:
            yield
        finally:
            n = lib.axon_stop_nrt_profile(str(output_dir).encode())
            print(f"ntff profile: {n} file(s) written to {output_dir}")

    holder["h"] = _hook


def run(inputs_array, trace=False, **kwargs):
    """inputs_array: [N, 3] float32. Returns (out [N] float32, BassKernelResults)."""
    import ml_dtypes

    pts = np.ascontiguousarray(inputs_array, dtype=np.float32)
    assert pts.shape == (N, 3), pts.shape
    # Host-side: a = |x| in f16 (SDF is sign-symmetric), then de-interleave
    # each tile to planar [P, 3, K] layout.
    a16 = np.abs(pts).astype(np.float16).reshape(NCORES, NPC, 3)
    if trace:
        _ensure_ntff_hook()
    nc = _get_nc()
    eye_bf = np.eye(P, dtype=np.float32).astype(ml_dtypes.bfloat16)
    in_maps = []
    for i in range(NCORES):
        m = {"eye": eye_bf}
        off = 0
        for t, k in enumerate(KS):
            blk = a16[i, off : off + P * k].reshape(P, k, 3)
            m[f"x{t}"] = np.ascontiguousarray(blk.transpose(0, 2, 1)).reshape(
                P, 3 * k
            )
            off += P * k
        in_maps.append(m)
    res = bass_utils.run_bass_kernel_spmd(
        nc, in_maps, core_ids=list(range(NCORES)), trace=trace, **kwargs
    )
    out = np.concatenate(
        [res.results[i][f"d{t}"].reshape(-1) for i in range(NCORES) for t in range(NT)]
    ).astype(np.float32)
    return out, res


def kernel(**inputs):
    out, _ = run(inputs["inputs"])
    return out


if __name__ == "__main__":
    rng = np.random.default_rng(0)
    pts = rng.standard_normal((N, 3)).astype(np.float32)
    out, _ = run(pts)
    q = np.abs(pts) - SIZE
    inside = np.all(q < 0, axis=1)
    d_out = np.sqrt(np.sum(np.square(np.maximum(q, 0.0)), axis=1))
    d_in = -np.max(q, axis=1)
    exp = np.where(inside, d_in, d_out)
    err = np.abs(out - exp) / np.maximum(np.abs(exp), 1e-6)
    print("max rel err:", err.max(), "mean:", err.mean())


# revision 14
# speedup vs baseline: 1.1419x; 1.1419x over previous
"""Box-SDF (CAPUDF box boundary distance) Trainium2 Bass kernel, v5.

For each 3-D point x (S = 0.4), with a = |x| (host-computed; the SDF is
sign-symmetric):
    q  = a - S
    d  = sqrt(sum_i relu(q_i)^2)    if any q_i >= 0   (outside)
    d  = -max_i q_i                 otherwise         (inside)

Select-free identity used on chip: the relu(q_0) plane and the inside
term (min(max_i a_i, S) - S) are never simultaneously nonzero, so they
merge exactly into ONE signed plane (squaring kills the sign):
    e0 = max(a_0 - S, min(max(a_1, a_2), S) - S)
         (= relu(q_0) outside, = max_i a_i - S < 0 inside)
    d  = sqrt( e0^2 + relu(q_1)^2 + relu(q_2)^2 )

On-chip dataflow per tile (planar f16 input [P, 3K], all contiguous;
pre = [e0 | b1 | b2], sq = pre^2 elementwise, split ACT/DVE at column U):
    DVE: q0  = TS(a0, add -S)                    (4x-mode tensor_scalar)
         b12 = TS([a1|a2], max S, add -S)
         m12 = TT(a1, a2, max)                   (2x-mode tensor_tensor)
         mc  = TS(m12, min S, add -S)
         e0  = TT(q0, mc, max)
         sq[U:3K]  = TT(pre * pre) -> bf16
    ACT: sq[0:U]   = Square(pre[0:U]) -> bf16    (one pass, no bias)
         d    = Sqrt(s_psum) -> f16              (same activation table set)
    PE : s = sq_e0 + sq1 + sq2 via identity-matmul PSUM accumulation
         (3 planes x K/512 chunks; eye stationary in bf16)
Tile sizes [1024, 2048, 2048, 2048, 1024] shorten the pipeline head
(first compute starts after a 0.5 MB DMA) and tail (small last B-stage).
f16/bf16 end-to-end halves HBM traffic vs f32; data-parallel on 8 cores.
"""

import sys

import numpy as np

sys.path.insert(0, "/opt/trn_rl_repo")

import concourse.bacc as bacc  # noqa: E402
import concourse.mybir as mybir  # noqa: E402
from concourse import bass_utils  # noqa: E402
from concourse.tile import TileContext  # noqa: E402

N = 8388608
NCORES = 8
NPC = N // NCORES  # 1,048,576 points per core
P = 128
KS = [1024, 2048, 2048, 2048, 1024]  # points per partition row, per tile
NT = len(KS)
assert P * sum(KS) == NPC

SIZE = 0.4
F16 = mybir.dt.float16
BF16 = mybir.dt.bfloat16
F32 = mybir.dt.float32
AF = mybir.ActivationFunctionType
OP = mybir.AluOpType


def build_kernel():
    nc = bacc.Bacc(
        "TRN2",
        target_bir_lowering=False,
        debug=False,
        num_devices=NCORES,
    )
    xs = [
        nc.dram_tensor(f"x{t}", [P, 3 * k], F16, kind="ExternalInput").ap()
        for t, k in enumerate(KS)
    ]
    eye = nc.dram_tensor("eye", [P, P], BF16, kind="ExternalInput").ap()
    ds = [
        nc.dram_tensor(f"d{t}", [P, k], F16, kind="ExternalOutput").ap()
        for t, k in enumerate(KS)
    ]

    with TileContext(nc) as tc:
        with (
            tc.tile_pool(name="const", bufs=1) as cpool,
            tc.tile_pool(name="xtp", bufs=4) as xtp,
            tc.tile_pool(name="pre", bufs=3) as prep,
            tc.tile_pool(name="sq", bufs=3) as sqp,
            tc.tile_pool(name="small", bufs=3) as small,
            tc.tile_pool(name="out", bufs=3) as outp,
            tc.tile_pool(name="psum", bufs=2, space="PSUM") as pspool,
        ):
            eye_t = cpool.tile([P, P], BF16)
            # Warm the Square/Sqrt activation table set while DMA ramps up.
            warm = cpool.tile([P, 8], F16)
            nc.vector.memset(warm[:], 0.0)
            nc.scalar.activation(out=warm[:], in_=warm[:], func=AF.Square)
            nc.scalar.activation(out=warm[:], in_=warm[:], func=AF.Sqrt)
            state = {}

            def stage_a(t):
                K = KS[t]
                xt = xtp.tile([P, 3 * K], F16, tag="xt")
                if t == 0:
                    # Chunk tile 0's DMA per plane so DVE starts sooner.
                    for c in range(3):
                        cs = slice(c * K, (c + 1) * K)
                        nc.sync.dma_start(out=xt[:, cs], in_=xs[t][:, cs])
                else:
                    nc.sync.dma_start(out=xt[:], in_=xs[t])

                a0, a1, a2 = (xt[:, c * K : (c + 1) * K] for c in range(3))
                U = 7 * K // 4  # ACT squares pre[0:U]; DVE squares pre[U:3K]
                # pre = [e0 | b1 | b2]
                pre = prep.tile([P, 3 * K], F16, tag="pre")
                # q0 = a0 - S (signed)
                q0 = small.tile([P, K], F16, tag="q0")
                nc.vector.tensor_scalar(
                    out=q0[:], in0=a0, scalar1=-SIZE, scalar2=None, op0=OP.add
                )
                # b12 = relu(a12 - S) = max(a12, S) - S
                nc.vector.tensor_scalar(
                    out=pre[:, K : 3 * K],
                    in0=xt[:, K : 3 * K],
                    scalar1=SIZE,
                    scalar2=-SIZE,
                    op0=OP.max,
                    op1=OP.add,
                )
                # m12 = max(a1, a2); mc = min(m12, S) - S
                m12 = small.tile([P, K], F16, tag="m12")
                nc.vector.tensor_tensor(out=m12[:], in0=a1, in1=a2, op=OP.max)
                mc = small.tile([P, K], F16, tag="mc")
                nc.vector.tensor_scalar(
                    out=mc[:],
                    in0=m12[:],
                    scalar1=SIZE,
                    scalar2=-SIZE,
                    op0=OP.min,
                    op1=OP.add,
                )
                # e0 = max(q0, mc): relu(q0) outside, max_i a_i - S inside
                nc.vector.tensor_tensor(
                    out=pre[:, 0:K], in0=q0[:], in1=mc[:], op=OP.max
                )

                # sq = pre^2 in bf16 (full-rate PE moving data):
                # ACT squares [0:U], DVE squares [U:3K]
                sq = sqp.tile([P, 3 * K], BF16, tag="sq")
                nc.vector.tensor_tensor(
                    out=sq[:, U : 3 * K],
                    in0=pre[:, U : 3 * K],
                    in1=pre[:, U : 3 * K],
                    op=OP.mult,
                )
                nc.scalar.activation(
                    out=sq[:, 0:U],
                    in_=pre[:, 0:U],
                    func=AF.Square,
                )
                state[t] = sq

            def stage_b(t):
                K = KS[t]
                sq = state.pop(t)
                # s = sq_e0 + sq1 + sq2 via identity matmuls accumulating in
                # PSUM (TensorE is otherwise idle; accumulate = free add)
                s_ps = pspool.tile([P, K], F32, tag="s_ps")
                dt = outp.tile([P, K], F16, tag="dt")
                last = t >= NT - 2
                for j in range(0, K, 512):
                    for c in range(3):
                        nc.tensor.matmul(
                            s_ps[:, j : j + 512],
                            eye_t[:],
                            sq[:, c * K + j : c * K + j + 512],
                            start=(c == 0),
                            stop=(c == 2),
                        )
                    if last:
                        # Tail tile: sqrt + store per 512-chunk so the
                        # final DMA overlaps the remaining matmul groups.
                        js = slice(j, j + 512)
                        nc.scalar.activation(
                            out=dt[:, js], in_=s_ps[:, js], func=AF.Sqrt
                        )
                        nc.gpsimd.dma_start(out=ds[t][:, js], in_=dt[:, js])
                if not last:
                    # d = sqrt(s)  (ScalarE reads PSUM directly, writes f16)
                    nc.scalar.activation(out=dt[:], in_=s_ps[:], func=AF.Sqrt)
                    nc.gpsimd.dma_start(out=ds[t], in_=dt[:])

            # 2-stage software pipeline emission: A(t+1) before B(t) so each
            # engine's in-order stream never stalls tile t+1's front work
            # behind tile t's tail work.
            stage_a(0)
            nc.sync.dma_start(out=eye_t[:], in_=eye[:])
            for t in range(1, NT):
                stage_b(t - 1)
                stage_a(t)
            stage_b(NT - 1)

    nc.compile()
    return nc


_cached_nc = None


def _get_nc():
    global _cached_nc
    if _cached_nc is None:
        _cached_nc = build_kernel()
    return _cached_nc


_AXON_SO = "/opt/axon/libaxon_pjrt.so"


def _ensure_ntff_hook():
    """Install an antenv.axon_hooks shim backed by libaxon_pjrt's NRT
    profiling C ABI, so run_bass_kernel_spmd(trace=True) works under axon."""
    try:
        from antenv.axon_hooks import get_axon_ntff_profile_hook  # noqa: F401

        return
    except ImportError:
        pass
    import contextlib
    import ctypes
    import types

    import antenv

    holder = {}
    mod = types.ModuleType("antenv.axon_hooks")
    mod.set_axon_ntff_profile_hook = lambda h: holder.__setitem__("h", h)
    mod.get_axon_ntff_profile_hook = lambda: holder.get("h")
    sys.modules["antenv.axon_hooks"] = mod
    antenv.axon_hooks = mod

    try:
        lib = ctypes.CDLL(_AXON_SO)
    except OSError:
        return
    if not hasattr(lib, "axon_start_nrt_profile"):
        return
    lib.axon_start_nrt_profile.argtypes = [
        ctypes.POINTER(ctypes.c_int64),
        ctypes.c_size_t,
    ]
    lib.axon_start_nrt_profile.restype = ctypes.c_int64
    lib.axon_stop_nrt_profile.argtypes = [ctypes.c_char_p]
    lib.axon_stop_nrt_profile.restype = ctypes.c_int64

    @contextlib.contextmanager
    def _hook(output_dir, device_ids):
        import jax

        jax.devices()
        if device_ids:
            ids = (ctypes.c_int64 * len(device_ids))(*device_ids)
            rc = lib.axon_start_nrt_profile(ids, len(device_ids))
        else:
            rc = lib.axon_start_nrt_profile(None, 0)
        if rc != 0:
            raise RuntimeError(f"axon_start_nrt_profile rc={rc}")
        try:
            yield
        finally:
            n = lib.axon_stop_nrt_profile(str(output_dir).encode())
            print(f"ntff profile: {n} file(s) written to {output_dir}")

    holder["h"] = _hook


def run(inputs_array, trace=False, **kwargs):
    """inputs_array: [N, 3] float32. Returns (out [N] float32, BassKernelResults)."""
    import ml_dtypes

    pts = np.ascontiguousarray(inputs_array, dtype=np.float32)
    assert pts.shape == (N, 3), pts.shape
    # Host-side: a = |x| in f16 (SDF is sign-symmetric), then de-interleave
    # each tile to planar [P, 3, K] layout.
    a16 = np.abs(pts).astype(np.float16).reshape(NCORES, NPC, 3)
    if trace:
        _ensure_ntff_hook()
    nc = _get_nc()
    eye_bf = np.eye(P, dtype=np.float32).astype(ml_dtypes.bfloat16)
    in_maps = []
    for i in range(NCORES):
        m = {"eye": eye_bf}
        off = 0
        for t, k in enumerate(KS):
            blk = a16[i, off : off + P * k].reshape(P, k, 3)
            m[f"x{t}"] = np.ascontiguousarray(blk.transpose(0, 2, 1)).reshape(
                P, 3 * k
            )
            off += P * k
        in_maps.append(m)
    res = bass_utils.run_bass_kernel_spmd(
        nc, in_maps, core_ids=list(range(NCORES)), trace=trace, **kwargs
    )
    out = np.concatenate(
        [res.results[i][f"d{t}"].reshape(-1) for i in range(NCORES) for t in range(NT)]
    ).astype(np.float32)
    return out, res


def kernel(**inputs):
    out, _ = run(inputs["inputs"])
    return out


if __name__ == "__main__":
    rng = np.random.default_rng(0)
    pts = rng.standard_normal((N, 3)).astype(np.float32)
    out, _ = run(pts)
    q = np.abs(pts) - SIZE
    inside = np.all(q < 0, axis=1)
    d_out = np.sqrt(np.sum(np.square(np.maximum(q, 0.0)), axis=1))
    d_in = -np.max(q, axis=1)
    exp = np.where(inside, d_in, d_out)
    err = np.abs(out - exp) / np.maximum(np.abs(exp), 1e-6)
    print("max rel err:", err.max(), "mean:", err.mean())


# revision 15
# speedup vs baseline: 1.1826x; 1.0356x over previous
"""Box-SDF (CAPUDF box boundary distance) Trainium2 Bass kernel, v5.

For each 3-D point x (S = 0.4), with a = |x| (host-computed; the SDF is
sign-symmetric):
    q  = a - S
    d  = sqrt(sum_i relu(q_i)^2)    if any q_i >= 0   (outside)
    d  = -max_i q_i                 otherwise         (inside)

Select-free identity used on chip: the relu(q_0) plane and the inside
term (min(max_i a_i, S) - S) are never simultaneously nonzero, so they
merge exactly into ONE signed plane (squaring kills the sign):
    e0 = max(a_0 - S, min(max(a_1, a_2), S) - S)
         (= relu(q_0) outside, = max_i a_i - S < 0 inside)
    d  = sqrt( e0^2 + relu(q_1)^2 + relu(q_2)^2 )

On-chip dataflow per tile (planar f16 input [P, 3K], all contiguous;
pre = [e0 | b1 | b2], sq = pre^2 elementwise, split ACT/DVE at column U):
    DVE: q0  = TS(a0, add -S)                    (4x-mode tensor_scalar)
         b12 = TS([a1|a2], max S, add -S)
         m12 = TT(a1, a2, max)                   (2x-mode tensor_tensor)
         mc  = TS(m12, min S, add -S)
         e0  = TT(q0, mc, max)
         sq[U:3K]  = TT(pre * pre) -> bf16
    ACT: sq[0:U]   = Square(pre[0:U]) -> bf16    (one pass, no bias)
         d    = Sqrt(s_psum) -> f16              (same activation table set)
    PE : s = sq_e0 + sq1 + sq2 via identity-matmul PSUM accumulation
         (3 planes x K/512 chunks; eye stationary in bf16)
Tile sizes [1024, 2048, 2048, 2048, 1024] shorten the pipeline head
(first compute starts after a 0.5 MB DMA) and tail (small last B-stage).
f16/bf16 end-to-end halves HBM traffic vs f32; data-parallel on 8 cores.
"""

import sys

import numpy as np

sys.path.insert(0, "/opt/trn_rl_repo")

import concourse.bacc as bacc  # noqa: E402
import concourse.mybir as mybir  # noqa: E402
from concourse import bass_utils  # noqa: E402
from concourse.tile import TileContext  # noqa: E402

N = 8388608
NCORES = 8
NPC = N // NCORES  # 1,048,576 points per core
P = 128
KS = [1024, 2048, 2048, 2048, 1024]  # points per partition row, per tile
NT = len(KS)
assert P * sum(KS) == NPC

SIZE = 0.4
F16 = mybir.dt.float16
BF16 = mybir.dt.bfloat16
F32 = mybir.dt.float32
AF = mybir.ActivationFunctionType
OP = mybir.AluOpType


def build_kernel():
    nc = bacc.Bacc(
        "TRN2",
        target_bir_lowering=False,
        debug=False,
        num_devices=NCORES,
    )
    xs = [
        nc.dram_tensor(f"x{t}", [P, 3 * k], F16, kind="ExternalInput").ap()
        for t, k in enumerate(KS)
    ]
    eye = nc.dram_tensor("eye", [P, P], BF16, kind="ExternalInput").ap()
    ds = [
        nc.dram_tensor(f"d{t}", [P, k], F16, kind="ExternalOutput").ap()
        for t, k in enumerate(KS)
    ]

    with TileContext(nc) as tc:
        with (
            tc.tile_pool(name="const", bufs=1) as cpool,
            tc.tile_pool(name="xtp", bufs=4) as xtp,
            tc.tile_pool(name="pre", bufs=3) as prep,
            tc.tile_pool(name="sq", bufs=3) as sqp,
            tc.tile_pool(name="small", bufs=3) as small,
            tc.tile_pool(name="out", bufs=3) as outp,
            tc.tile_pool(name="psum", bufs=2, space="PSUM") as pspool,
        ):
            eye_t = cpool.tile([P, P], BF16)
            # Warm the Square/Sqrt activation table set while DMA ramps up.
            warm = cpool.tile([P, 8], F16)
            nc.vector.memset(warm[:], 0.0)
            nc.scalar.activation(out=warm[:], in_=warm[:], func=AF.Square)
            nc.scalar.activation(out=warm[:], in_=warm[:], func=AF.Sqrt)
            state = {}

            def stage_a(t):
                K = KS[t]
                xt = xtp.tile([P, 3 * K], F16, tag="xt")
                if t == 0:
                    # Chunk tile 0's DMA per plane so DVE starts sooner.
                    for c in range(3):
                        cs = slice(c * K, (c + 1) * K)
                        nc.sync.dma_start(out=xt[:, cs], in_=xs[t][:, cs])
                else:
                    nc.sync.dma_start(out=xt[:], in_=xs[t])

                a0, a1, a2 = (xt[:, c * K : (c + 1) * K] for c in range(3))
                U = 2 * K  # ACT squares pre[0:U]; DVE squares pre[U:3K]
                # pre = [e0 | b1 | b2]
                pre = prep.tile([P, 3 * K], F16, tag="pre")
                # q0 = a0 - S (signed)
                q0 = small.tile([P, K], F16, tag="q0")
                nc.vector.tensor_scalar(
                    out=q0[:], in0=a0, scalar1=-SIZE, scalar2=None, op0=OP.add
                )
                # b12 = relu(a12 - S) = max(a12, S) - S
                nc.vector.tensor_scalar(
                    out=pre[:, K : 3 * K],
                    in0=xt[:, K : 3 * K],
                    scalar1=SIZE,
                    scalar2=-SIZE,
                    op0=OP.max,
                    op1=OP.add,
                )
                # m12 = max(a1, a2); mc = min(m12, S) - S
                m12 = small.tile([P, K], F16, tag="m12")
                nc.vector.tensor_tensor(out=m12[:], in0=a1, in1=a2, op=OP.max)
                mc = small.tile([P, K], F16, tag="mc")
                nc.vector.tensor_scalar(
                    out=mc[:],
                    in0=m12[:],
                    scalar1=SIZE,
                    scalar2=-SIZE,
                    op0=OP.min,
                    op1=OP.add,
                )
                # e0 = max(q0, mc): relu(q0) outside, max_i a_i - S inside
                nc.vector.tensor_tensor(
                    out=pre[:, 0:K], in0=q0[:], in1=mc[:], op=OP.max
                )

                # sq = pre^2 in bf16 (full-rate PE moving data):
                # ACT squares [0:U], DVE squares [U:3K]
                sq = sqp.tile([P, 3 * K], BF16, tag="sq")
                nc.vector.tensor_tensor(
                    out=sq[:, U : 3 * K],
                    in0=pre[:, U : 3 * K],
                    in1=pre[:, U : 3 * K],
                    op=OP.mult,
                )
                nc.scalar.activation(
                    out=sq[:, 0:U],
                    in_=pre[:, 0:U],
                    func=AF.Square,
                )
                state[t] = sq

            def stage_b(t):
                K = KS[t]
                sq = state.pop(t)
                # s = sq_e0 + sq1 + sq2 via identity matmuls accumulating in
                # PSUM (TensorE is otherwise idle; accumulate = free add)
                s_ps = pspool.tile([P, K], F32, tag="s_ps")
                dt = outp.tile([P, K], F16, tag="dt")
                last = t == NT - 1
                for j in range(0, K, 512):
                    for c in range(3):
                        nc.tensor.matmul(
                            s_ps[:, j : j + 512],
                            eye_t[:],
                            sq[:, c * K + j : c * K + j + 512],
                            start=(c == 0),
                            stop=(c == 2),
                        )
                    if last:
                        # Tail tile: sqrt + store per 512-chunk so the
                        # final DMA overlaps the remaining matmul groups.
                        js = slice(j, j + 512)
                        nc.scalar.activation(
                            out=dt[:, js], in_=s_ps[:, js], func=AF.Sqrt
                        )
                        nc.gpsimd.dma_start(out=ds[t][:, js], in_=dt[:, js])
                if not last:
                    # d = sqrt(s)  (ScalarE reads PSUM directly, writes f16)
                    nc.scalar.activation(out=dt[:], in_=s_ps[:], func=AF.Sqrt)
                    nc.gpsimd.dma_start(out=ds[t], in_=dt[:])

            # 2-stage software pipeline emission: A(t+1) before B(t) so each
            # engine's in-order stream never stalls tile t+1's front work
            # behind tile t's tail work.
            stage_a(0)
            nc.sync.dma_start(out=eye_t[:], in_=eye[:])
            for t in range(1, NT):
                stage_b(t - 1)
                stage_a(t)
            stage_b(NT - 1)

    nc.compile()
    return nc


_cached_nc = None


def _get_nc():
    global _cached_nc
    if _cached_nc is None:
        _cached_nc = build_kernel()
    return _cached_nc


_AXON_SO = "/opt/axon/libaxon_pjrt.so"


def _ensure_ntff_hook():
    """Install an antenv.axon_hooks shim backed by libaxon_pjrt's NRT
    profiling C ABI, so run_bass_kernel_spmd(trace=True) works under axon."""
    try:
        from antenv.axon_hooks import get_axon_ntff_profile_hook  # noqa: F401

        return
    except ImportError:
        pass
    import contextlib
    import ctypes
    import types

    import antenv

    holder = {}
    mod = types.ModuleType("antenv.axon_hooks")
    mod.set_axon_ntff_profile_hook = lambda h: holder.__setitem__("h", h)
    mod.get_axon_ntff_profile_hook = lambda: holder.get("h")
    sys.modules["antenv.axon_hooks"] = mod
    antenv.axon_hooks = mod

    try:
        lib = ctypes.CDLL(_AXON_SO)
    except OSError:
        return
    if not hasattr(lib, "axon_start_nrt_profile"):
        return
    lib.axon_start_nrt_profile.argtypes = [
        ctypes.POINTER(ctypes.c_int64),
        ctypes.c_size_t,
    ]
    lib.axon_start_nrt_profile.restype = ctypes.c_int64
    lib.axon_stop_nrt_profile.argtypes = [ctypes.c_char_p]
    lib.axon_stop_nrt_profile.restype = ctypes.c_int64

    @contextlib.contextmanager
    def _hook(output_dir, device_ids):
        import jax

        jax.devices()
        if device_ids:
            ids = (ctypes.c_int64 * len(device_ids))(*device_ids)
            rc = lib.axon_start_nrt_profile(ids, len(device_ids))
        else:
            rc = lib.axon_start_nrt_profile(None, 0)
        if rc != 0:
            raise RuntimeError(f"axon_start_nrt_profile rc={rc}")
        try:
            yield
        finally:
            n = lib.axon_stop_nrt_profile(str(output_dir).encode())
            print(f"ntff profile: {n} file(s) written to {output_dir}")

    holder["h"] = _hook


def run(inputs_array, trace=False, **kwargs):
    """inputs_array: [N, 3] float32. Returns (out [N] float32, BassKernelResults)."""
    import ml_dtypes

    pts = np.ascontiguousarray(inputs_array, dtype=np.float32)
    assert pts.shape == (N, 3), pts.shape
    # Host-side: a = |x| in f16 (SDF is sign-symmetric), then de-interleave
    # each tile to planar [P, 3, K] layout.
    a16 = np.abs(pts).astype(np.float16).reshape(NCORES, NPC, 3)
    if trace:
        _ensure_ntff_hook()
    nc = _get_nc()
    eye_bf = np.eye(P, dtype=np.float32).astype(ml_dtypes.bfloat16)
    in_maps = []
    for i in range(NCORES):
        m = {"eye": eye_bf}
        off = 0
        for t, k in enumerate(KS):
            blk = a16[i, off : off + P * k].reshape(P, k, 3)
            m[f"x{t}"] = np.ascontiguousarray(blk.transpose(0, 2, 1)).reshape(
                P, 3 * k
            )
            off += P * k
        in_maps.append(m)
    res = bass_utils.run_bass_kernel_spmd(
        nc, in_maps, core_ids=list(range(NCORES)), trace=trace, **kwargs
    )
    out = np.concatenate(
        [res.results[i][f"d{t}"].reshape(-1) for i in range(NCORES) for t in range(NT)]
    ).astype(np.float32)
    return out, res


def kernel(**inputs):
    out, _ = run(inputs["inputs"])
    return out


if __name__ == "__main__":
    rng = np.random.default_rng(0)
    pts = rng.standard_normal((N, 3)).astype(np.float32)
    out, _ = run(pts)
    q = np.abs(pts) - SIZE
    inside = np.all(q < 0, axis=1)
    d_out = np.sqrt(np.sum(np.square(np.maximum(q, 0.0)), axis=1))
    d_in = -np.max(q, axis=1)
    exp = np.where(inside, d_in, d_out)
    err = np.abs(out - exp) / np.maximum(np.abs(exp), 1e-6)
    print("max rel err:", err.max(), "mean:", err.mean())
